# revision 5
# baseline (speedup 1.0000x reference)
"""GwcVolumeCostProcessor Trainium2 kernel (v3).

Builds the groupwise-correlation + concat cost volume:
  out[1, 64, 48, 128, 240] f32 from
  ref_gwc/tgt_gwc [1, 320, 128, 240] and ref_concat/tgt_concat [1, 12, 128, 240].

Sharding: H axis (128 = 8 cores x 16 rows). The disparity shift is along W
only, so each core needs just its own 16-row slice of every input.

All 64 output channels ride one pipeline. The concat channels are folded in
as pseudo-products with identity weight columns:
  - gwc groups:  prod = ref[c] * tgt[c],     weights 1/8 block-diagonal
  - ref_concat:  prod = refc[i] * ones,      weights identity (A-side slice
                 [d:W] applies the w>=d masking for free)
  - tgt_concat:  prod = ones * tgtc[i],      weights identity (S-side slice
                 [0:wv] + psum dst [d:W] applies the shift for free)

Per-core pipeline (for each disparity d, descending):
  - DVE:   products for ch 0..255 in one fused [128,2,...] op + most of t2
  - Pool:  a small h-slice of the t2 product (DVE offload)
  - PE :   3 block matmuls x 8 psum-bank chunks -> PSUM partitions 0:96
  - ACT:   drains PSUM -> f32 staging (w<d strip stays zero: descending d)
  - DMA:   3 large per-d stores (16/16/32 channels x 15KB descriptors) on
           the sync HWDGE ring, the ACT HWDGE ring, and the gpsimd SWDGE
           queue so all three DMA streams run in parallel.
"""

import numpy as np
import ml_dtypes

C = 320          # gwc channels
G = 40           # groups
CPG = 8          # channels per group
D = 48           # disparity bins
H = 128          # full height
W = 240          # width
CC = 12          # concat channels per tensor
COUT = G + 2 * CC  # 64 output channels
NCORES = 8
HS = H // NCORES  # 16 rows per core

PSUM_P = 96   # psum/staging partition extent
T2_ROWS = 88  # t2: 64 gwc ch + 12 refc + 12 ones
GP_H = 4      # h-rows of the t2 product computed on GpSimd instead of DVE

_CACHE = {}


def _make_weights():
    """Per-tile stationary matrices, bf16."""
    w0 = np.zeros((128, 16), dtype=np.float32)
    for r in range(128):
        w0[r, r // CPG] = 1.0 / CPG
    w1 = w0.copy()
    w2 = np.zeros((T2_ROWS, 32), dtype=np.float32)
    for r in range(64):
        w2[r, r // CPG] = 1.0 / CPG          # gwc groups 32..39 -> cols 0..7
    for i in range(CC):
        w2[64 + i, 8 + i] = 1.0              # ref_concat -> cols 8..19
        w2[76 + i, 20 + i] = 1.0             # tgt_concat -> cols 20..31
    return [w.astype(ml_dtypes.bfloat16) for w in (w0, w1, w2)]


def _build_nc():
    from concourse import bacc, mybir
    import concourse.tile as tile

    f32 = mybir.dt.float32
    bf16 = mybir.dt.bfloat16

    nc = bacc.Bacc("TRN2", target_bir_lowering=False, debug=False)

    ref = nc.dram_tensor("ref_gwc", [C, HS, W], f32, kind="ExternalInput")
    tgt = nc.dram_tensor("tgt_gwc", [C, HS, W], f32, kind="ExternalInput")
    refc = nc.dram_tensor("ref_concat", [CC, HS, W], f32, kind="ExternalInput")
    tgtc = nc.dram_tensor("tgt_concat", [CC, HS, W], f32, kind="ExternalInput")
    wd = [
        nc.dram_tensor("w0", [128, 16], bf16, kind="ExternalInput"),
        nc.dram_tensor("w1", [128, 16], bf16, kind="ExternalInput"),
        nc.dram_tensor("w2", [T2_ROWS, 32], bf16, kind="ExternalInput"),
    ]
    out = nc.dram_tensor("out", [COUT, D, HS, W], f32, kind="ExternalOutput")

    with tile.TileContext(nc) as tc:
        _kernel_body(nc, tc, ref, tgt, refc, tgtc, wd, out, mybir)

    nc.compile()
    return nc


def _kernel_body(nc, tc, ref, tgt, refc, tgtc, wd, out, mybir):
    f32 = mybir.dt.float32
    bf16 = mybir.dt.bfloat16
    out_ap = out.ap()

    with (
        tc.tile_pool(name="const", bufs=1) as constp,
        tc.tile_pool(name="prod", bufs=2) as prodp,
        tc.tile_pool(name="psum", bufs=2, space="PSUM") as psump,
    ):
        # --- weights ---
        wt = []
        for t, shp in enumerate([(128, 16), (128, 16), (T2_ROWS, 32)]):
            w_t = constp.tile(list(shp), bf16, name=f"wt{t}", tag=f"wt{t}")
            nc.sync.dma_start(w_t[:], wd[t].ap())
            wt.append(w_t)

        # --- input tiles (bf16; cast inside the SWDGE DMA) ---
        # Fused t0+t1 tiles: [128, 2, HS, W]; slab 0 = ch 0..127, slab 1 =
        # ch 128..255. t2: [88, HS, W] = gwc ch 256..319 + refc + ones.
        # A side sliced [d:W] in the loop; B side = A shifted one element
        # (data at [..., 1:W+1]) so odd-d slices stay 4-byte aligned for
        # DVE 2x; S side sliced [0:wv].
        a01 = constp.tile([128, 2, HS, W], bf16, name="a01", tag="a01")
        t01 = constp.tile([128, 2, HS, W], bf16, name="t01", tag="t01")
        nc.gpsimd.dma_start(a01[:, 0], ref[0:128])
        nc.gpsimd.dma_start(a01[:, 1], ref[128:256])
        nc.gpsimd.dma_start(t01[:, 0], tgt[0:128])
        nc.gpsimd.dma_start(t01[:, 1], tgt[128:256])
        b01 = constp.tile([128, 2, HS, W + 4], bf16, name="b01", tag="b01")
        nc.scalar.copy(b01[:, :, :, 1:W + 1], a01[:])

        a2 = constp.tile([T2_ROWS, HS, W], bf16, name="a2", tag="a2")
        t2 = constp.tile([T2_ROWS, HS, W], bf16, name="t2", tag="t2")
        nc.gpsimd.dma_start(a2[0:64], ref[256:320])
        nc.gpsimd.dma_start(t2[0:64], tgt[256:320])
        # memset partition base must be 32-aligned: set ones over [64:88],
        # then the concat loads overwrite their half (WAW, program order)
        nc.gpsimd.memset(a2[64:88], 1.0)
        nc.gpsimd.memset(t2[64:88], 1.0)
        nc.gpsimd.dma_start(a2[64:76], refc.ap())      # refc rows
        nc.gpsimd.dma_start(t2[76:88], tgtc.ap())      # tgtc rows
        b2 = constp.tile([T2_ROWS, HS, W + 4], bf16, name="b2", tag="b2")
        nc.scalar.copy(b2[:, :, 1:W + 1], a2[:])

        # staging buffers (3-slot rotation; zeroed once, then the
        # descending-d order keeps the w<d strip zero forever)
        stg = []
        for i in range(3):
            s = constp.tile([PSUM_P, HS, W], f32, name=f"stg{i}", tag=f"stg{i}")
            nc.gpsimd.memset(s[:], 0.0)
            stg.append(s)

        # --- main disparity loop (descending) ---
        for di, d in enumerate(reversed(range(D))):
            wv = W - d
            s = stg[di % 3]
            hk = HS - GP_H  # DVE h-rows of t2

            # products (bf16): ch 0..255 in one DVE op; t2 split DVE/Pool
            p01 = prodp.tile([128, 2, HS, W], bf16, name=f"p01_{d}", tag="p01")
            p2 = prodp.tile([T2_ROWS, HS, W], bf16, name=f"p2_{d}", tag="p2")
            if d % 2 == 0:
                r01 = a01[:, :, :, d:W]
                r2a = a2[:, 0:hk, d:W]
                r2b = a2[:, hk:HS, d:W]
            else:
                r01 = b01[:, :, :, d + 1:W + 1]
                r2a = b2[:, 0:hk, d + 1:W + 1]
                r2b = b2[:, hk:HS, d + 1:W + 1]
            nc.vector.tensor_mul(p01[:, :, :, 0:wv], r01, t01[:, :, :, 0:wv])
            nc.vector.tensor_mul(p2[:, 0:hk, 0:wv], r2a, t2[:, 0:hk, 0:wv])
            nc.gpsimd.tensor_mul(p2[:, hk:HS, 0:wv], r2b, t2[:, hk:HS, 0:wv])

            # group-reduce on PE, drain on ACT, one h-half at a time
            for hh in range(2):
                ps = psump.tile([PSUM_P, HS // 2, 256], f32,
                                name=f"ps_{d}_{hh}", tag="ps")
                for k in range(4):
                    h0 = hh * 8 + 2 * k
                    nc.tensor.matmul(
                        ps[0:16, 2 * k:2 * k + 2, d:W],
                        wt[0][:, :],
                        p01[:, 0, h0:h0 + 2, 0:wv],
                        start=True, stop=True,
                    )
                    nc.tensor.matmul(
                        ps[32:48, 2 * k:2 * k + 2, d:W],
                        wt[1][:, :],
                        p01[:, 1, h0:h0 + 2, 0:wv],
                        start=True, stop=True,
                    )
                    nc.tensor.matmul(
                        ps[64:96, 2 * k:2 * k + 2, d:W],
                        wt[2][:, :],
                        p2[:, h0:h0 + 2, 0:wv],
                        start=True, stop=True,
                    )
                nc.scalar.copy(s[:, hh * 8:hh * 8 + 8, d:W], ps[:, :, d:W])

            # per-d stores: 3 large DMAs on 3 independent DMA streams
            # psum/staging partition map: 0:16 -> ch 0:16, 32:48 -> ch 16:32,
            # 64:96 -> ch 32:64 (gwc 32..39, refc, tgtc)
            nc.sync.dma_start(out_ap[0:16, d], s[0:16])
            nc.scalar.dma_start(out_ap[16:32, d], s[32:48])
            nc.gpsimd.dma_start(out_ap[32:64, d], s[64:96])


def _get_nc():
    if "nc" not in _CACHE:
        _CACHE["nc"] = _build_nc()
    return _CACHE["nc"]


def kernel(ref_gwc, tgt_gwc, ref_concat, tgt_concat):
    from concourse.bass_utils import run_bass_kernel_spmd

    ref_gwc = np.asarray(ref_gwc, dtype=np.float32)
    tgt_gwc = np.asarray(tgt_gwc, dtype=np.float32)
    ref_concat = np.asarray(ref_concat, dtype=np.float32)
    tgt_concat = np.asarray(tgt_concat, dtype=np.float32)

    nc = _get_nc()
    ws = _make_weights()

    in_maps = []
    for i in range(NCORES):
        sl = slice(i * HS, (i + 1) * HS)
        m = {
            "ref_gwc": np.ascontiguousarray(ref_gwc[0, :, sl, :]),
            "tgt_gwc": np.ascontiguousarray(tgt_gwc[0, :, sl, :]),
            "ref_concat": np.ascontiguousarray(ref_concat[0, :, sl, :]),
            "tgt_concat": np.ascontiguousarray(tgt_concat[0, :, sl, :]),
        }
        for t, w in enumerate(ws):
            m[f"w{t}"] = w
        in_maps.append(m)

    res = run_bass_kernel_spmd(nc, in_maps, list(range(NCORES))).results

    full = np.empty((1, COUT, D, H, W), dtype=np.float32)
    for i in range(NCORES):
        full[0, :, :, i * HS:(i + 1) * HS, :] = res[i]["out"]
    return full
